# revision 2
# baseline (speedup 1.0000x reference)
import numpy as np
import jax
from jax.sharding import Mesh, PartitionSpec
from jax.experimental.shard_map import shard_map
import concourse.bass as bass
import concourse.mybir as mybir
from concourse.tile import TileContext
from concourse.bass2jax import (
    _bass_exec_p,
    install_neuronx_cc_hook,
    partition_id_tensor,
)

NCORES = 8
GRP = 512
OFFSETS = [(dx, dy, dz) for dx in (-1, 0, 1) for dy in (-1, 0, 1)
           for dz in (-1, 0, 1)]


def _patch_drain():
    from concourse.vector_clock import ScopedClock, VectorClock

    def _drain_and_barrier(self, tick_clock, wait_clock):
        gc = tick_clock.global_clock
        n = len(gc)
        idxs = [i for i in range(n) if gc[i] > 0]
        if not idxs:
            self.nc.sync.drain()
        for i in idxs:
            vec = [0] * n
            vec[i] = gc[i]
            drain_inst = self.nc.sync.drain()
            wait_clock.add_sem_waits(
                drain_inst.ins, ScopedClock({None: VectorClock(vec)})
            )
        self.nc.all_engine_barrier()
        assert self.sems is not None
        popped = self.nc._tile_sem_poison_stack.pop()
        assert popped is self._sem_poison
        self.nc.clear_and_free_semaphores(list(self.sems.allocated().values()))
        self.nc.all_engine_barrier()

    TileContext._drain_and_barrier = _drain_and_barrier


def _patch_bir_wait_split():
    # this walrus build rejects >1 sync wait per instruction: split extras
    # onto NoOp instructions placed immediately before on the same engine
    import json as _json
    import concourse.bass2jax as b2j
    if getattr(b2j, "_wait_split_patched", False):
        return
    orig = b2j.compile_bir_kernel

    def split_waits(bir_bytes):
        b = _json.loads(bir_bytes)
        nnop = [0]
        for fn in b.get("functions", []):
            for blk in fn.get("blocks", []):
                out = []
                for ins in blk.get("instructions", []):
                    si = ins.get("sync_info") or {}
                    ow = si.get("on_wait") or []
                    if len(ow) > 1:
                        for w in ow[:-1]:
                            nnop[0] += 1
                            out.append({
                                "name": f"I-nopw{nnop[0]}",
                                "opcode": "NoOp",
                                "engine": ins["engine"],
                                "ins": [], "outs": [],
                                "is_reset_sema": False,
                                "debug": ins.get("debug", 0),
                                "sync_info": {"on_update": [],
                                              "on_wait": [w]},
                            })
                        si["on_wait"] = [ow[-1]]
                    out.append(ins)
                blk["instructions"] = out
        return _json.dumps(b).encode()

    def wrapped(bir_json, tmpdir, neff_name="file.neff"):
        return orig(split_waits(bir_json), tmpdir, neff_name=neff_name)

    b2j.compile_bir_kernel = wrapped
    b2j._wait_split_patched = True


_patch_drain()
_patch_bir_wait_split()


def _pjrt_compile(nc, n_cores):
    install_neuronx_cc_hook()
    partition_name = (nc.partition_id_tensor.name
                      if nc.partition_id_tensor else None)
    in_names, out_names, out_avals, zero_shapes = [], [], [], []
    for alloc in nc.m.functions[0].allocations:
        if not isinstance(alloc, mybir.MemoryLocationSet):
            continue
        name = alloc.memorylocations[0].name
        if alloc.kind == "ExternalInput":
            if name != partition_name:
                in_names.append(name)
        elif alloc.kind == "ExternalOutput":
            out_names.append(name)
            shape = tuple(alloc.tensor_shape)
            dtype = mybir.dt.np(alloc.dtype)
            out_avals.append(jax.core.ShapedArray(shape, dtype))
            zero_shapes.append((shape, dtype))
    n_params = len(in_names)
    n_outs = len(out_avals)
    in_names_all = (in_names + out_names
                    + ([partition_name] if partition_name else []))

    def _body(*args):
        operands = list(args)
        if partition_name is not None:
            operands.append(partition_id_tensor())
        return tuple(_bass_exec_p.bind(
            *operands, out_avals=tuple(out_avals),
            in_names=tuple(in_names_all), out_names=tuple(out_names),
            lowering_input_output_aliases=(),
            sim_require_finite=False, sim_require_nnan=False, nc=nc))

    donate = tuple(range(n_params, n_params + n_outs))
    devices = jax.devices()[:n_cores]
    mesh = Mesh(np.asarray(devices), ("core",))
    sharded = jax.jit(
        shard_map(_body, mesh=mesh,
                  in_specs=(PartitionSpec("core"),) * (n_params + n_outs),
                  out_specs=(PartitionSpec("core"),) * n_outs,
                  check_rep=False),
        donate_argnums=donate, keep_unused=True)

    def run(in_maps):
        per_core = [[np.ascontiguousarray(m[name]) for name in in_names]
                    for m in in_maps]
        concat_in = [np.concatenate([per_core[c][i] for c in range(n_cores)],
                                    axis=0) for i in range(n_params)]
        concat_zeros = [np.zeros((n_cores * s[0], *s[1:]), d)
                        for s, d in zero_shapes]
        out_arrs = jax.block_until_ready(sharded(*concat_in, *concat_zeros))
        return [{name: np.asarray(out_arrs[i]).reshape(
                    n_cores, *out_avals[i].shape)[c]
                 for i, name in enumerate(out_names)}
                for c in range(n_cores)]

    return run


def _build_layer_kernel(slabs, KT, C_out, M, has_pos):
    f32r = mybir.dt.float32r
    f32 = mybir.dt.float32
    nc = bass.Bass()
    at_d = nc.dram_tensor("at", [KT, M], f32r, kind="ExternalInput")
    w_d = nc.dram_tensor("wstk", [KT, C_out], f32r, kind="ExternalInput")
    if has_pos:
        pos_d = nc.dram_tensor("pose", [C_out, M], f32, kind="ExternalInput")
    ft_d = nc.dram_tensor("ft", [C_out, M], f32, kind="ExternalOutput")
    n_oc = (C_out + 127) // 128
    G = len(slabs)
    abufs = 3 if M > GRP else 1
    with TileContext(nc) as tc:
        with tc.tile_pool(name="w", bufs=1) as wp, \
             tc.tile_pool(name="a", bufs=abufs) as ap, \
             tc.tile_pool(name="o", bufs=3) as op, \
             tc.tile_pool(name="ps", bufs=4, space="PSUM") as pp:
            w_sbs = []
            for g, (r0, rc) in enumerate(slabs):
                w_sb = wp.tile([rc, C_out], f32r, name=f"w{g}")
                nc.sync.dma_start(out=w_sb[:], in_=w_d[r0:r0 + rc, :])
                w_sbs.append(w_sb)
            pos_sbs = []
            if has_pos:
                for oc in range(n_oc):
                    o0 = oc * 128
                    ow = min(128, C_out - o0)
                    ps = wp.tile([ow, M], f32, name=f"pos{oc}")
                    nc.sync.dma_start(out=ps[:], in_=pos_d[o0:o0 + ow, :])
                    pos_sbs.append(ps)
            for m0 in range(0, M, GRP):
                a_sbs = []
                for g, (r0, rc) in enumerate(slabs):
                    a_sb = ap.tile([rc, GRP], f32r, name=f"a{g}", tag=f"a{g}")
                    nc.sync.dma_start(out=a_sb[:],
                                      in_=at_d[r0:r0 + rc, m0:m0 + GRP])
                    a_sbs.append(a_sb)
                for oc in range(n_oc):
                    o0 = oc * 128
                    ow = min(128, C_out - o0)
                    p = pp.tile([ow, GRP], f32, name="p", tag="p")
                    for g in range(G):
                        nc.tensor.matmul(out=p[:],
                                         lhsT=w_sbs[g][:, o0:o0 + ow],
                                         rhs=a_sbs[g][:],
                                         start=(g == 0), stop=(g == G - 1))
                    o_sb = op.tile([ow, GRP], f32, name="o", tag="o")
                    if has_pos:
                        nc.vector.tensor_tensor(
                            out=o_sb[:], in0=p[:],
                            in1=pos_sbs[oc][:, m0:m0 + GRP],
                            op=mybir.AluOpType.add)
                    else:
                        nc.vector.tensor_copy(out=o_sb[:], in_=p[:])
                    nc.sync.dma_start(out=ft_d[o0:o0 + ow, m0:m0 + GRP],
                                      in_=o_sb[:])
    return _pjrt_compile(nc, NCORES)


_RUNNERS = {}


def _layer_runner(KT, C_out, M, has_pos, first):
    key = (KT, C_out, M, has_pos)
    if key not in _RUNNERS:
        slabs = ([(0, KT)] if first
                 else [(g * 128, 128) for g in range(KT // 128)])
        _RUNNERS[key] = _build_layer_kernel(slabs, KT, C_out, M, has_pos)
    return _RUNNERS[key]


def _keys_np(coords, D):
    c = coords.astype(np.int64)
    return (c[:, 0] * D + c[:, 1]) * D + c[:, 2]


def _rulebook(coords_in, coords_out, D_in):
    keys_in = _keys_np(coords_in, D_in)
    n_in = keys_in.shape[0]
    base = coords_out.astype(np.int64) * 2
    off = np.array(OFFSETS, dtype=np.int64)
    q = base[:, None, :] + off[None, :, :]
    valid = np.all((q >= 0) & (q < D_in), axis=2)
    qc = np.clip(q, 0, D_in - 1)
    qk = (qc[:, :, 0] * D_in + qc[:, :, 1]) * D_in + qc[:, :, 2]
    idx = np.minimum(np.searchsorted(keys_in, qk), n_in - 1)
    hit = valid & (keys_in[idx] == qk)
    return idx.astype(np.int64), hit


def _run_layer(F_in, coords_in, coords_out, W, D_in, pos_rows=None):
    n_out = coords_out.shape[0]
    C_in = W.shape[1]
    C_out = W.shape[2]
    first = C_in == 1
    KT = 27 if first else 27 * C_in + (-27 * C_in) % 128
    base_rows = (n_out + NCORES - 1) // NCORES
    M = base_rows + (-base_rows) % GRP

    idx, hit = _rulebook(coords_in, coords_out, D_in)

    wstk = np.zeros((KT, C_out), np.float32)
    for k in range(27):
        wstk[k * C_in:(k + 1) * C_in, :] = W[k]

    in_maps = []
    spans = []
    for c in range(NCORES):
        j0 = c * base_rows
        j1 = min(j0 + base_rows, n_out)
        n = max(j1 - j0, 0)
        spans.append((j0, j1, n))
        AT = np.zeros((KT, M), np.float32)
        if n > 0:
            g = F_in[idx[j0:j1]]                      # [n, 27, C_in]
            g[~hit[j0:j1]] = 0.0
            AT[:27 * C_in, :n] = g.reshape(n, 27 * C_in).T
        m = {"at": AT, "wstk": wstk}
        if pos_rows is not None:
            pe = np.zeros((C_out, M), np.float32)
            if n > 0:
                pe[:, :n] = pos_rows[j0:j1].T
            m["pose"] = pe
        in_maps.append(m)

    run = _layer_runner(KT, C_out, M, pos_rows is not None, first)
    res = run(in_maps)
    F_out = np.empty((n_out, C_out), np.float32)
    for c, (j0, j1, n) in enumerate(spans):
        if n > 0:
            F_out[j0:j1] = res[c]["ft"][:, :n].T
    return F_out


def kernel(features, W1, W2, W3, W4, W5, pos_emb,
           coords0, coords1, coords2, coords3, coords4, coords5,
           **_unused):
    features = np.asarray(features, np.float32)
    Ws = [np.asarray(w, np.float32) for w in (W1, W2, W3, W4, W5)]
    coords = [np.asarray(c) for c in (coords0, coords1, coords2,
                                      coords3, coords4, coords5)]
    pos_emb = np.asarray(pos_emb, np.float32)
    f = features
    D = 512
    for l in range(5):
        pos_rows = pos_emb[:coords[l + 1].shape[0]] if l == 4 else None
        f = _run_layer(f, coords[l], coords[l + 1], Ws[l], D,
                       pos_rows=pos_rows)
        D //= 2
    return f
